# revision 13
# baseline (speedup 1.0000x reference)
"""AttentionBlock (GroupNorm32 + qkv 1x1 + channel-attention + proj + residual)
for Trainium2, SPMD over 8 NeuronCores (data-parallel over batch B=8).

Self-contained: hardcodes shapes B=8, C=1024, L=4096, H=16, groups=32.
kernel(**inputs) takes the FULL numpy inputs and returns the FULL output.

Math per batch b (all on one core):
  xn    = groupnorm(x) * gn_w + gn_b          (stats via bn_stats + PE group reduce)
  qkT   = xn^T @ Wqk^T (scale folded in)      [L, 2C] transposed orientation
  score = qT_h^T kT_h accumulated over L      [64, 64] per head, PSUM-resident
  w     = softmax(score, -1); wT via PE transpose, packed block-diagonal 2 heads
  v     = Wv xn + vb    (natural orientation, recomputed per L chunk)
  ctx   = wT2 @ v       (block-diag trick: 2 heads per [128,128] matmul)
  out   = xn + Wp ctx + pb
Matmuls run in float32r (tf32, full PE rate at N>=256).
"""

import os
import sys

try:
    import concourse.bass  # noqa: F401
except ImportError:  # pragma: no cover
    sys.path.insert(0, "/opt/trn_rl_repo")

import numpy as np

import concourse.bass as bass
import concourse.bacc as bacc
import concourse.tile as tile
from concourse import mybir
from concourse.bass_utils import run_bass_kernel_spmd

B, C, L, H = 8, 1024, 4096, 16
G = 32          # groupnorm groups
CH = C // H     # 64 channels per head
EPS = 1e-5
CT = C // 128   # 8 channel tiles
NLB = L // 512  # 8 l-blocks of 512
F32 = mybir.dt.float32
F32R = mybir.dt.float32r

Alu = mybir.AluOpType
Act = mybir.ActivationFunctionType


def _build():
    nc = bacc.Bacc("TRN2", target_bir_lowering=False, debug=False, num_devices=8)

    x = nc.declare_dram_parameter("x", [C, L], F32, isOutput=False)
    wqkt = nc.declare_dram_parameter("wqkt", [C, 2 * C], F32R, isOutput=False)
    qkb = nc.declare_dram_parameter("qkb", [2 * C], F32, isOutput=False)
    wvt = nc.declare_dram_parameter("wvt", [C, C], F32R, isOutput=False)
    vb = nc.declare_dram_parameter("vb", [128, CT], F32, isOutput=False)
    wpt = nc.declare_dram_parameter("wpt", [C, C], F32R, isOutput=False)
    pb = nc.declare_dram_parameter("pb", [128, CT], F32, isOutput=False)
    gnw = nc.declare_dram_parameter("gnw", [128, CT], F32, isOutput=False)
    gnb = nc.declare_dram_parameter("gnb", [128, CT], F32, isOutput=False)
    gsel = nc.declare_dram_parameter("gsel", [128, 4], F32, isOutput=False)
    gbr = nc.declare_dram_parameter("gbr", [4, 128], F32, isOutput=False)
    ident = nc.declare_dram_parameter("ident", [128, 64], F32, isOutput=False)
    out = nc.declare_dram_parameter("out", [C, L], F32, isOutput=True)

    with tile.TileContext(nc) as tc:
        _body(nc, tc, x, wqkt, qkb, wvt, vb, wpt, pb, gnw, gnb, gsel, gbr, ident, out)
    nc.compile()
    return nc


def _body(nc, tc, x, wqkt, qkb, wvt, vb, wpt, pb, gnw, gnb, gsel, gbr, ident, out):
    from contextlib import ExitStack

    with ExitStack() as ctx:
        singles = ctx.enter_context(tc.tile_pool(name="singles", bufs=1))

        # ---- persistent small tiles -------------------------------------
        gsel_sb = singles.tile([128, 4], F32, name="gsel")
        nc.sync.dma_start(out=gsel_sb, in_=gsel[:, :])
        gbr_sb = singles.tile([4, 128], F32, name="gbr")
        nc.sync.dma_start(out=gbr_sb, in_=gbr[:, :])
        ident_sb = singles.tile([128, 64], F32, name="ident")
        nc.sync.dma_start(out=ident_sb, in_=ident[:, :])
        gnw_sb = singles.tile([128, CT], F32, name="gnw")
        nc.sync.dma_start(out=gnw_sb, in_=gnw[:, :])
        gnb_sb = singles.tile([128, CT], F32, name="gnb")
        nc.sync.dma_start(out=gnb_sb, in_=gnb[:, :])
        vb_sb = singles.tile([128, CT], F32, name="vb")
        nc.sync.dma_start(out=vb_sb, in_=vb[:, :])
        pb_sb = singles.tile([128, CT], F32, name="pb")
        nc.sync.dma_start(out=pb_sb, in_=pb[:, :])
        eps_sb = singles.tile([128, 1], F32, name="eps")
        nc.vector.memset(eps_sb, EPS)
        scale_sb = singles.tile([128, CT], F32, name="scale")
        bias_sb = singles.tile([128, CT], F32, name="biasc")

        # block-diagonal softmax-transpose tiles (2 heads each), filled later
        wt2_sb = [singles.tile([128, 128], F32R, name=f"wt2_{j}")
                  for j in range(H // 2)]

        # ---- stage A: groupnorm statistics ------------------------------
        with tc.tile_pool(name="stA", bufs=2) as pa, \
             tc.tile_pool(name="psA", bufs=1, space="PSUM") as pps:
            tall = singles.tile([128, 2 * CT], F32, name="tall")
            for ct in range(CT):
                xt = pa.tile([128, L], F32, name="xa")
                nc.sync.dma_start(out=xt, in_=x[ct * 128:(ct + 1) * 128, :])
                xr = xt.rearrange("p (n f) -> p n f", f=512)
                st = pa.tile([128, L // 512, 6], F32, name="bnst")
                for sg in range(L // 512):
                    nc.vector.bn_stats(out=st[:, sg, :], in_=xr[:, sg, :])
                mv = pa.tile([128, 2], F32, name="mv")
                nc.vector.bn_aggr(out=mv, in_=st)
                # tall columns: 2ct -> mean, 2ct+1 -> E[x^2]
                nc.vector.tensor_copy(out=tall[:, 2 * ct:2 * ct + 1], in_=mv[:, 0:1])
                msq = pa.tile([128, 1], F32, name="msq")
                nc.vector.tensor_mul(out=msq, in0=mv[:, 0:1], in1=mv[:, 0:1])
                nc.vector.tensor_add(out=tall[:, 2 * ct + 1:2 * ct + 2],
                                     in0=mv[:, 1:2], in1=msq)
            # cross-partition reduce within 32-channel groups (matmul w/ selector)
            gst_ps = pps.tile([4, 2 * CT], F32, name="gst")
            nc.tensor.matmul(out=gst_ps, lhsT=gsel_sb, rhs=tall, start=True, stop=True)
            gst_sb = pa.tile([4, 2 * CT], F32, name="gstsb")
            nc.vector.tensor_scalar_mul(out=gst_sb, in0=gst_ps, scalar1=1.0 / 32.0)
            # broadcast group stats back to channels (matmul w/ broadcast selector)
            chst_ps = pps.tile([128, 2 * CT], F32, name="chst")
            nc.tensor.matmul(out=chst_ps, lhsT=gbr_sb, rhs=gst_sb, start=True, stop=True)
            ch = chst_ps.rearrange("p (t two) -> p t two", two=2)
            mu = pa.tile([128, CT], F32, name="mu")
            nc.vector.tensor_copy(out=mu, in_=ch[:, :, 0])
            var = pa.tile([128, CT], F32, name="var")
            nc.vector.tensor_mul(out=var, in0=mu, in1=mu)
            nc.vector.tensor_sub(out=var, in0=ch[:, :, 1], in1=var)
            nc.scalar.activation(out=var, in_=var, func=Act.Sqrt,
                                 bias=eps_sb, scale=1.0)
            nc.vector.reciprocal(out=var, in_=var)          # rstd
            nc.vector.tensor_mul(out=scale_sb, in0=var, in1=gnw_sb)
            nc.vector.tensor_mul(out=var, in0=mu, in1=scale_sb)
            nc.vector.tensor_sub(out=bias_sb, in0=gnb_sb, in1=var)

        # ---- stage B: qk projection (transposed) + score accumulation ---
        # Scores are packed 2 q-heads x 4 k-heads per matmul: lhsT is a
        # head-pair of q columns, rhs a 256-wide slab of k columns (N=256
        # keeps fp32r at full PE rate); only the per-head diagonal 64x64
        # blocks are used. The x-block pools are shared with stage C so
        # chunk prefetch crosses the stage boundary without a pool barrier.
        vw = ctx.enter_context(tc.tile_pool(name="vw", bufs=1))
        wvt_sb = [vw.tile([128, C], F32R, name=f"wvt{ct}") for ct in range(CT)]
        pxb = ctx.enter_context(tc.tile_pool(name="pxb", bufs=2))
        pxn = ctx.enter_context(tc.tile_pool(name="pxn", bufs=2))

        def load_xblock(lb):
            xb = pxb.tile([128, CT, 512], F32, name="xb")
            for ct in range(CT):
                nc.scalar.dma_start(
                    out=xb[:, ct, :],
                    in_=x[ct * 128:(ct + 1) * 128, lb * 512:(lb + 1) * 512])
            xn = pxn.tile([128, CT, 512], F32R, name="xnb")
            for ct in range(CT):
                nc.gpsimd.tensor_scalar(
                    out=xn[:, ct, :], in0=xb[:, ct, :],
                    scalar1=scale_sb[:, ct:ct + 1], scalar2=bias_sb[:, ct:ct + 1],
                    op0=Alu.mult, op1=Alu.add)
            return xb, xn

        psoft = ctx.enter_context(tc.tile_pool(name="soft", bufs=1))
        with tc.tile_pool(name="scps", bufs=1, space="PSUM") as scps:
            scoreq = [scps.tile([128, 512], F32, name=f"scoreq{g}")
                      for g in range(4)]

            def emit_score(q, lt):
                for j in range(H // 2):
                    g = j // 2
                    nc.tensor.matmul(
                        out=scoreq[g][:, (j % 2) * 256:(j % 2) * 256 + 256],
                        lhsT=q[:, j * 128:(j + 1) * 128],
                        rhs=q[:, C + g * 256:C + (g + 1) * 256],
                        start=(lt == 0 and j % 2 == 0), stop=(lt == L // 128 - 1),
                        skip_group_check=True)

            with tc.tile_pool(name="qkw", bufs=1) as pw, \
                 tc.tile_pool(name="stB", bufs=2) as pbf, \
                 tc.tile_pool(name="qkps", bufs=2, space="PSUM") as qkps:
                wqkt_sb = []
                for oc in range(4):
                    for ct in range(CT):
                        if oc == 0:
                            wqkt_sb.append(
                                pw.tile([128, 2 * C], F32R, name=f"wqk{ct}"))
                        nc.scalar.dma_start(
                            out=wqkt_sb[ct][:, oc * 512:(oc + 1) * 512],
                            in_=wqkt[ct * 128:(ct + 1) * 128,
                                     oc * 512:(oc + 1) * 512])
                # qk bias broadcast to all partitions via stride-0 DMA
                qkb_sb = pw.tile([128, 2 * C], F32, name="qkb")
                qkb_ap = qkb[:]
                qkb_bc = bass.AP(tensor=qkb_ap.tensor, offset=qkb_ap.offset,
                                 ap=[[0, 128]] + list(qkb_ap.ap))
                nc.sync.dma_start(out=qkb_sb, in_=qkb_bc)

                pending = None
                for lb in range(NLB):
                    xb, xnb = load_xblock(lb)
                    if lb == NLB - 1:
                        xb_last, xnb_last = xb, xnb
                    if lb == 1:
                        # v-projection weights: needed from the softmax
                        # transition onward; emitted here so their DMA does
                        # not compete with x/wqk loads in the prefix
                        for ct in range(CT):
                            nc.sync.dma_start(
                                out=wvt_sb[ct],
                                in_=wvt[ct * 128:(ct + 1) * 128, :])
                    for sub in range(4):
                        lt = lb * 4 + sub
                        qkt = pbf.tile([128, 2 * C], F32R, name="qkt")
                        for oc in range(4):
                            ps = qkps.tile([128, 512], F32, name="qkp")
                            for ct in range(CT):
                                nc.tensor.matmul(
                                    out=ps,
                                    lhsT=xnb[:, ct, sub * 128:(sub + 1) * 128],
                                    rhs=wqkt_sb[ct][:, oc * 512:(oc + 1) * 512],
                                    start=(ct == 0), stop=(ct == CT - 1))
                            nc.vector.tensor_add(
                                out=qkt[:, oc * 512:(oc + 1) * 512], in0=ps,
                                in1=qkb_sb[:, oc * 512:(oc + 1) * 512])
                        if pending is not None:
                            emit_score(*pending)
                        pending = (qkt, lt)
                emit_score(*pending)

            # ---- softmax + per-head transpose ---------------------------
            # head h = pair j=h//2, odd=h%2: score block lives in
            # scoreq[j//2] at partitions odd*64, cols (j%2)*384 + odd*64
            negmax = psoft.tile([128, H // 2], F32, name="negmax")
            sumexp = psoft.tile([128, H // 2], F32, name="sumexp")
            exp_sb = psoft.tile([128, 512], F32, name="expsb")
            w_sb = psoft.tile([128, 512], F32, name="wsb")
            rs = psoft.tile([128, H // 2], F32, name="rsum")

            def _blk(h):
                j, odd = h // 2, h % 2
                bank = scoreq[j // 2]
                p0 = odd * 64
                c0 = (j % 2) * 384 + odd * 64
                return j, odd, bank, p0, c0

            for h in range(H):
                j, odd, bank, p0, c0 = _blk(h)
                nc.vector.tensor_reduce(
                    out=negmax[p0:p0 + 64, j:j + 1],
                    in_=bank[p0:p0 + 64, c0:c0 + 64],
                    axis=mybir.AxisListType.X, op=Alu.max, negate=True)
            for h in range(H):
                j, odd, bank, p0, c0 = _blk(h)
                nc.scalar.activation(
                    out=exp_sb[p0:p0 + 64, j * 64:(j + 1) * 64],
                    in_=bank[p0:p0 + 64, c0:c0 + 64], func=Act.Exp,
                    bias=negmax[p0:p0 + 64, j:j + 1], scale=1.0,
                    accum_out=sumexp[p0:p0 + 64, j:j + 1])
            nc.vector.reciprocal(out=rs, in_=sumexp)
            for h in range(H):
                j, odd, bank, p0, c0 = _blk(h)
                nc.vector.tensor_scalar_mul(
                    out=w_sb[p0:p0 + 64, j * 64:(j + 1) * 64],
                    in0=exp_sb[p0:p0 + 64, j * 64:(j + 1) * 64],
                    scalar1=rs[p0:p0 + 64, j:j + 1])
            # zero the block-diagonal tiles (memset can't write f32r)
            zsrc = psoft.tile([128, 128], F32, name="zsrc")
            nc.vector.memset(zsrc, 0.0)
            for j in range(H // 2):
                nc.vector.tensor_copy(out=wt2_sb[j], in_=zsrc)
            # odd heads live at partitions 64:128; shift their w down via a
            # small SBUF->SBUF DMA so the (partition-0-only) transpose
            # matmuls can consume them
            wodd = psoft.tile([64, 512], F32, name="wodd")
            for j in range(H // 2):
                nc.sync.dma_start(out=wodd[:, j * 64:(j + 1) * 64],
                                  in_=w_sb[64:128, j * 64:(j + 1) * 64])

        def build_wt2():
            # PE transposes + quadrant placement; emitted between chunk-0's
            # v-matmuls and its ctx-matmuls so the PE never idles waiting on
            # the softmax chain.
            wtf = psoft.tile([64, 1024], F32R, name="wtf")
            with tc.tile_pool(name="trps", bufs=2, space="PSUM") as trps:
                for j in range(H // 2):
                    tp = trps.tile([64, 64], F32, name="wtp")
                    nc.tensor.transpose(out=tp,
                                        in_=w_sb[0:64, j * 64:(j + 1) * 64],
                                        identity=ident_sb[0:64, :])
                    nc.vector.tensor_copy(out=wtf[:, j * 128:j * 128 + 64],
                                          in_=tp)
                    tp2 = trps.tile([64, 64], F32, name="wtp")
                    nc.tensor.transpose(out=tp2,
                                        in_=wodd[:, j * 64:(j + 1) * 64],
                                        identity=ident_sb[0:64, :])
                    nc.vector.tensor_copy(
                        out=wtf[:, j * 128 + 64:j * 128 + 128], in_=tp2)
            for j in range(H // 2):
                nc.vector.tensor_copy(out=wt2_sb[j][0:64, 0:64],
                                      in_=wtf[:, j * 128:j * 128 + 64])
                nc.sync.dma_start(out=wt2_sb[j][64:128, 64:128],
                                  in_=wtf[:, j * 128 + 64:j * 128 + 128])

        # ---- stage C: v, ctx, proj, residual ----------------------------
        with tc.tile_pool(name="cw", bufs=1) as pw2, \
             tc.tile_pool(name="stC", bufs=2) as pc, \
             tc.tile_pool(name="ctxp", bufs=2) as pctx, \
             tc.tile_pool(name="cps", bufs=2, space="PSUM") as cps:
            wpt_sb = []
            for ct in range(CT):
                w = pw2.tile([128, C], F32R, name=f"wpt{ct}")
                nc.sync.dma_start(out=w, in_=wpt[ct * 128:(ct + 1) * 128, :])
                wpt_sb.append(w)
            for idx, lc in enumerate([NLB - 1] + list(range(NLB - 1))):
                if lc == NLB - 1:
                    xc, xn = xb_last, xnb_last   # still resident from stage B
                else:
                    xc, xn = load_xblock(lc)
                v_sb = pc.tile([128, CT, 512], F32R, name="vsb")
                for ot in range(CT):
                    ps = cps.tile([128, 512], F32, name="vps")
                    for ct in range(CT):
                        nc.tensor.matmul(
                            out=ps,
                            lhsT=wvt_sb[ct][:, ot * 128:(ot + 1) * 128],
                            rhs=xn[:, ct, :],
                            start=(ct == 0), stop=(ct == CT - 1))
                    nc.vector.tensor_scalar_add(out=v_sb[:, ot, :], in0=ps,
                                                scalar1=vb_sb[:, ot:ot + 1])
                if idx == 0:
                    build_wt2()
                ctx_sb = pctx.tile([128, CT, 512], F32R, name="ctxsb")
                for j in range(CT):
                    ps = cps.tile([128, 512], F32, name="cxps")
                    nc.tensor.matmul(out=ps, lhsT=wt2_sb[j],
                                     rhs=v_sb[:, j, :], start=True, stop=True)
                    nc.vector.tensor_copy(out=ctx_sb[:, j, :], in_=ps)
                for ot in range(CT):
                    ps = cps.tile([128, 512], F32, name="hps")
                    for ct in range(CT):
                        nc.tensor.matmul(
                            out=ps,
                            lhsT=wpt_sb[ct][:, ot * 128:(ot + 1) * 128],
                            rhs=ctx_sb[:, ct, :],
                            start=(ct == 0), stop=(ct == CT - 1))
                    # out = (h + proj_bias) + xn   (in-place into the x tile)
                    # NOTE: xn read natively as f32r -- a .bitcast() AP clones
                    # the Tile handle and escapes Tile dependency tracking.
                    nc.vector.scalar_tensor_tensor(
                        out=xc[:, ot, :], in0=ps, scalar=pb_sb[:, ot:ot + 1],
                        in1=xn[:, ot, :], op0=Alu.add, op1=Alu.add)
                    nc.sync.dma_start(
                        out=out[ot * 128:(ot + 1) * 128, lc * 512:(lc + 1) * 512],
                        in_=xc[:, ot, :])


_NC_CACHE = {}


def _get_nc():
    if "nc" not in _NC_CACHE:
        _NC_CACHE["nc"] = _build()
    return _NC_CACHE["nc"]


def _round_tf32(x):
    u = x.view(np.uint32).copy()
    lsb = (u >> 13) & np.uint32(1)
    u = u + np.uint32(0x0FFF) + lsb
    u &= np.uint32(0xFFFFE000)
    return u.view(np.float32)


def _host_prep(x, gn_w, gn_b, qkv_w, qkv_b, proj_w, proj_b):
    s = np.float32(1.0 / np.sqrt(np.sqrt(CH)))
    # reference splits qkv PER HEAD: channel block h*192..(h+1)*192 = [q_h|k_h|v_h]
    qw = qkv_w.reshape(H, 3, CH, C)
    qb3 = qkv_b.reshape(H, 3, CH)
    wq = np.ascontiguousarray(qw[:, 0].reshape(C, C))    # head-major q rows
    wk = np.ascontiguousarray(qw[:, 1].reshape(C, C))
    wv = np.ascontiguousarray(qw[:, 2].reshape(C, C))
    bq = np.ascontiguousarray(qb3[:, 0].reshape(C))
    bk = np.ascontiguousarray(qb3[:, 1].reshape(C))
    bv = np.ascontiguousarray(qb3[:, 2].reshape(C))
    wqk = (np.concatenate([wq, wk], axis=0) * s).astype(np.float32)  # fold attn scale
    qkb_h = (np.concatenate([bq, bk]) * s).astype(np.float32)
    wqkt = _round_tf32(np.ascontiguousarray(wqk.T))       # [C, 2C]
    wvt = _round_tf32(np.ascontiguousarray(wv.T))         # [C, C]
    vb_h = np.ascontiguousarray(bv.reshape(CT, 128).T)    # [128, CT]
    wpt = _round_tf32(np.ascontiguousarray(proj_w.T))     # [C, C]
    pb_h = np.ascontiguousarray(proj_b.reshape(CT, 128).T)
    gnw_h = np.ascontiguousarray(gn_w.reshape(CT, 128).T)
    gnb_h = np.ascontiguousarray(gn_b.reshape(CT, 128).T)
    gsel_h = np.zeros((128, 4), np.float32)
    for p in range(128):
        gsel_h[p, p // 32] = 1.0
    gbr_h = np.ascontiguousarray(gsel_h.T)
    ident_h = np.vstack([np.eye(64, dtype=np.float32)] * 2)
    base = {
        "wqkt": wqkt, "qkb": qkb_h, "wvt": wvt, "vb": vb_h,
        "wpt": wpt, "pb": pb_h, "gnw": gnw_h, "gnb": gnb_h,
        "gsel": gsel_h, "gbr": gbr_h, "ident": ident_h,
    }
    in_maps = []
    for b in range(B):
        m = dict(base)
        m["x"] = np.ascontiguousarray(x[b])
        in_maps.append(m)
    return in_maps


def kernel(x, gn_w, gn_b, qkv_w, qkv_b, proj_w, proj_b):
    nc = _get_nc()
    in_maps = _host_prep(np.asarray(x, np.float32), np.asarray(gn_w, np.float32),
                         np.asarray(gn_b, np.float32), np.asarray(qkv_w, np.float32),
                         np.asarray(qkv_b, np.float32), np.asarray(proj_w, np.float32),
                         np.asarray(proj_b, np.float32))
    trace = bool(int(os.environ.get("ATT_TRACE", "0")))
    kwargs = {}
    if trace:
        kwargs = {"trace": True, "tmpdir": os.environ.get("ATT_TRACE_DIR", None)}
    res = run_bass_kernel_spmd(nc, in_maps, list(range(B)), **kwargs)
    out = np.stack([res.results[i]["out"] for i in range(B)], axis=0)
    if trace:
        kernel.last_exec_time_ns = res.exec_time_ns
    return out


kernel.last_exec_time_ns = None
